# revision 37
# baseline (speedup 1.0000x reference)
"""Trainium2 Bass kernel for nn_CrossAttention (sparse cross-attention).

Math (reference):
    q = xF @ Wq;  k = context @ Wk;  v = context @ Wv
    attn = softmax(scale * q k^T) over K keys
    out = (attn v) @ Wo + bo + xF, rows >= lengths[b] zeroed

Algebraic restructure (context has only 4 channels, so per-head k/v are
rank-4):
    Wqk[ch, h, e] = scale * sum_d Wq[ch, hD+d] Wk[e, hD+d]
    sT_h[key,row] = ctx[key,:] . (Wqk_h^T xF_row)         (PE, contraction 4)
    avT[(h,e), row] = sum_k exp(sT_h) [ctx;1][k,e]        (PE, contraction 128)
    out = Wvo^T (avT / lam) + bo + xF,   Wvo_h = Wv_h Wo_h

Single-pass interleaved head layout: all 8 heads live in one 128-partition
tile.  qk components at partitions 32g+4p+e (head 2g+p); av/avn at
partitions 16h+e (e<4: value comps, e=4: softmax normalizer from a ones
column).  Score matmuls are 4-way row-banded (strip g), av matmuls 4-way
col-banded; banded matmuls at disjoint tile positions execute concurrently
on the PE array, so emission batches them band-rotated.

Engine balance per supertile (R=512 rows, measured on hw): the Act-engine
exp (16 strips x 512 cols, ~8.4us with overheads) and the PE stream are
co-critical.  The PE executes ~540ns batch slots in which matmuls at
disjoint 32-wide tile positions run concurrently, so work is batched for
maximal in-slot concurrency: scores in 6 groups of [3,3,3,3,2,2] strips
(bounded by PSUM: 2 ping-pong slots x 3 banks), av in 4 quads of 4
col-bands (accumulation flags span the quads), qk as two 2-chunk
accumulation runs, out as 4 spread chunks sharing the qk PSUM bank
(WAR-paced by the DVE residual adds).  The softmax normalization is an
all-DVE chain reading av straight from PSUM (stream_shuffle of the
ones-row, fast reciprocal, multiply) so avn is ready early and gpsimd
stays free for DMA posting.  Inputs DMA 3 tiles ahead (xft on the sync
queue ring, ctx/c5 on the gpsimd ring, one descriptor set per tensor --
finer splits choke the ring on descriptor rate); output leaves per
128-channel chunk on the gpsimd ring.  Emission order is a priority hint
to the tile list-scheduler; this ordering is a tuned local optimum
(reorderings and priority boosts measured worse).
"""

import numpy as np

NUM_HEAD = 8
CH_HEAD = 64
CH = 512
CONTEXT_CH = 4
B, L, K = 16, 4096, 256
R = 512
N_CORES = 8

# exp groups: (slot 'A'|'B', strip start, strip end)
GROUPS = [("A", 12, 14), ("B", 14, 16), ("A", 0, 3), ("B", 3, 6),
          ("A", 6, 9), ("B", 9, 12)]


def _strip_info(s):
    g, wave = s % 4, s // 4
    return g, wave // 2, wave % 2  # row/col strip, parity, key chunk


def _build_host_constants(Wq, Wk, Wv, Wo, bo):
    scale = CH_HEAD ** (-0.5)
    Wq_h = Wq.reshape(CH, NUM_HEAD, CH_HEAD)
    Wk_h = Wk.reshape(CONTEXT_CH, NUM_HEAD, CH_HEAD)
    Wqk = scale * np.einsum("chd,ehd->che", Wq_h, Wk_h)  # [512, 8, 4]

    wqk = np.zeros((128, 4, 128), np.float16)  # [ch_part, chunk, m]
    for g in range(4):
        for par in range(2):
            h = 2 * g + par
            for c in range(4):
                wqk[:, c, 32 * g + 4 * par: 32 * g + 4 * par + 4] = \
                    Wqk[128 * c: 128 * (c + 1), h, :]

    Wv_h = Wv.reshape(CONTEXT_CH, NUM_HEAD, CH_HEAD)
    Wo_h = Wo.reshape(NUM_HEAD, CH_HEAD, CH)
    wvo = np.zeros((128, CH), np.float16)
    for h in range(NUM_HEAD):
        wvo[16 * h: 16 * h + 4, :] = Wv_h[:, h, :] @ Wo_h[h]
        wvo[16 * h + 4, :] = bo / NUM_HEAD  # avn row 16h+4 == 1.0 exactly
    return wqk, wvo


def _build_context(context):
    ctx = np.zeros((B, 128, 2, 256), np.float16)   # [...][:128]=A, [128:]=B
    c5 = np.zeros((B, 128, 2, 64), np.float16)     # [...][:32]=A, [32:]=B
    for b in range(B):
        cT = context[b].T  # [4, 256]
        for g in range(4):
            for kc in range(2):
                ctx[b, 32 * g: 32 * g + 4, kc, 0:128] = cT[:, 128 * kc:128 * (kc + 1)]
                ctx[b, 32 * g + 4: 32 * g + 8, kc, 128:256] = cT[:, 128 * kc:128 * (kc + 1)]
        for kc in range(2):
            c5[b, :, kc, 0:4] = context[b, 128 * kc:128 * (kc + 1), :]
            c5[b, :, kc, 4] = 1.0
            c5[b, :, kc, 32 + 16:32 + 20] = context[b, 128 * kc:128 * (kc + 1), :]
            c5[b, :, kc, 32 + 20] = 1.0
    return ctx, c5


def _build_program(T):
    import concourse.bass as bass  # noqa: F401
    import concourse.tile as tile
    from concourse import bacc, mybir

    f32 = mybir.dt.float32
    f16 = mybir.dt.float16
    Exp = mybir.ActivationFunctionType.Exp

    nc = bacc.Bacc("TRN2", target_bir_lowering=False, debug=False)

    xft_d = nc.dram_tensor("xft", [T, 128, 4, R], f16, kind="ExternalInput").ap()
    ctx_d = nc.dram_tensor("ctx", [T, 128, 2, 256], f16, kind="ExternalInput").ap()
    c5_d = nc.dram_tensor("c5", [T, 128, 2, 64], f16, kind="ExternalInput").ap()
    wqk_d = nc.dram_tensor("wqk", [128, 4, 128], f16, kind="ExternalInput").ap()
    wvo_d = nc.dram_tensor("wvo", [128, CH], f16, kind="ExternalInput").ap()
    out_d = nc.dram_tensor("outt", [T, 128, 4, R], f16, kind="ExternalOutput").ap()

    with tile.TileContext(nc) as tc:
        consts = tc.alloc_tile_pool(name="consts", bufs=1)
        wqk_s = consts.tile([128, 4, 128], f16)
        wvo_s = consts.tile([128, CH], f16)
        expbias = consts.tile([128, 1], f32)
        nc.vector.memset(expbias, -4.0)
        nc.scalar.dma_start(out=wqk_s, in_=wqk_d)
        nc.scalar.dma_start(out=wvo_s, in_=wvo_d)
        # Warm up the Act engine's Exp table during the DMA fill window;
        # otherwise the first real exp pays the ~1.3us ACT_TABLE_LOAD inline.
        actwarm = consts.tile([128, 1], f32)
        nc.scalar.activation(out=actwarm, in_=expbias, func=Exp)

        io = tc.alloc_tile_pool(name="io", bufs=5)
        ctxp = tc.alloc_tile_pool(name="ctxp", bufs=4)
        exp_pool = tc.alloc_tile_pool(name="exp_pool", bufs=2)
        workp = tc.alloc_tile_pool(name="workp", bufs=2)
        outsb = tc.alloc_tile_pool(name="outsb", bufs=2)
        ps_sc = tc.alloc_tile_pool(name="ps_sc", bufs=1, space="PSUM")
        ps_av = tc.alloc_tile_pool(name="ps_av", bufs=1, space="PSUM")
        ps_o = tc.alloc_tile_pool(name="ps_o", bufs=1, space="PSUM")

        # per-iteration live state
        xft = [None] * T
        ctxt = [None] * T
        c5t = [None] * T
        qk_sb = [None] * T
        ex = [None] * T
        av_ps = [None] * T
        avn = [None] * T
        out_sb = [None] * T
        sc_tiles = {}  # (t, group) -> psum tile

        def dma_in(t):
            # xft on the sync queue; ctx/c5 on the gpsimd queue
            xft[t] = io.tile([128, 4, R], f16, tag="xft", name="xft")
            nc.sync.dma_start(out=xft[t], in_=xft_d[t])
            ctxt[t] = ctxp.tile([128, 2, 256], f16, tag="ctx", name="ctx")
            nc.gpsimd.dma_start(out=ctxt[t], in_=ctx_d[t])
            c5t[t] = ctxp.tile([128, 2, 64], f16, tag="c5", name="c5")
            nc.gpsimd.dma_start(out=c5t[t], in_=c5_d[t])

        def alloc_sc(t, k):
            slot = GROUPS[k][0]
            sc_tiles[(t, k)] = ps_sc.tile([128, 3, R], f32, tag=f"sc{slot}",
                                          name=f"sc{slot}")
            return sc_tiles[(t, k)]

        def emit_sc_group(t, k):
            """Score matmuls for exp group k of tile t (row-banded)."""
            sc = sc_tiles.get((t, k))
            if sc is None:
                sc = alloc_sc(t, k)
            _, s0, s1 = GROUPS[k]
            for i, s in enumerate(range(s0, s1)):
                g, par, kc = _strip_info(s)
                nc.tensor.matmul(
                    out=sc[:, i, :],
                    lhsT=ctxt[t][32 * g: 32 * g + 8, kc,
                                 128 * par: 128 * (par + 1)],
                    rhs=qk_sb[t][32 * g: 32 * g + 8, :],
                    tile_position=(32 * g, 0),
                )

        def emit_exp(t, k):
            if t == 0 and k == 0:
                ex[t] = exp_pool.tile([128, 16, R], f16, tag="ex", name="ex")
            _, s0, s1 = GROUPS[k]
            n = s1 - s0
            nc.scalar.activation(
                out=ex[t][:, s0:s1, :],
                in_=sc_tiles[(t, k)][:, 0:n, :],
                func=Exp,
                bias=expbias,
            )

        qk_ps = [None] * T

        def emit_qk(t, c0, c1):
            """qk projection chunks [c0,c1) for tile t (shared out/qk bank)."""
            if c0 == 0:
                qk_ps[t] = ps_o.tile([128, R], f32, tag="o", name="qk_ps")
            for c in range(c0, c1):
                nc.tensor.matmul(
                    out=qk_ps[t],
                    lhsT=wqk_s[:, c, :],
                    rhs=xft[t][:, c, :],
                    start=(c == 0),
                    stop=(c == 3),
                )
            if c1 == 4:
                qk_sb[t] = workp.tile([128, R], f16, tag="qk_sb", name="qk_sb")
                nc.vector.tensor_copy(out=qk_sb[t], in_=qk_ps[t])

        def emit_av(t, w):
            """av matmuls for strip quad w (4 col-banded, concurrent).

            Execution order of quads is w3, w0, w1, w2 (following the
            rotated exp-group order), so accumulation starts on w3's strips
            and stops on w2's."""
            if w == 3:
                av_ps[t] = ps_av.tile([128, R], f32, tag="avlam", name="av_ps")
            for s in range(4 * w, 4 * w + 4):
                g, par, kc = _strip_info(s)
                nc.tensor.matmul(
                    out=av_ps[t][32 * g: 32 * (g + 1), :],
                    lhsT=c5t[t][:, kc, 32 * par: 32 * par + 32],
                    rhs=ex[t][:, s, :],
                    tile_position=(0, 32 * g),
                    start=(s >= 12),
                    stop=(8 <= s < 12),
                )

        def emit_norm(t):
            # all-DVE, av read straight from PSUM (shortest avn latency)
            lam_sb = workp.tile([128, R], f32, tag="lam_sb")
            avn[t] = workp.tile([128, R], f16, tag="avn", name="avn")
            lr_sb = workp.tile([128, R], f32, tag="lr_sb")
            nc.vector.stream_shuffle(out=lam_sb, in_=av_ps[t],
                                     mask=[4] * 16 + [20] * 16)
            nc.vector.reciprocal_approx_fast(out=lr_sb, in_=lam_sb)
            nc.vector.tensor_mul(avn[t], av_ps[t], lr_sb)

        def emit_out_chunk(t, c, pingpong=False, borrow=None):
            """Out-projection chunk c of tile t.

            pingpong (epilogue only): odd chunks use the freed av bank so
            the tail out chain isn't WAR-paced on a single PSUM bank.
            borrow=(tile, group): steady state, write into the idle 3rd
            bank of a 2-strip score group's slot instead of the shared o
            bank, breaking the o-bank WAR chain between consecutive chunks.
            """
            if c == 0:
                out_sb[t] = outsb.tile([128, 4, R], f16, tag="out_sb",
                                       name="out_sb")
            if borrow is not None:
                o_ps = sc_tiles[borrow][:, 2, :]
            elif pingpong and c % 2:
                o_ps = ps_av.tile([128, R], f32, tag="avlam", name="o2_ps")
            else:
                o_ps = ps_o.tile([128, R], f32, tag="o", name="o_ps")
            nc.tensor.matmul(
                out=o_ps,
                lhsT=wvo_s[:, 128 * c: 128 * (c + 1)],
                rhs=avn[t],
            )
            nc.vector.tensor_add(out_sb[t][:, c, :], o_ps, xft[t][:, c, :])
            nc.gpsimd.dma_start(out=out_d[t][:, c, :], in_=out_sb[t][:, c, :])

        # ---- prologue ----
        dma_in(0)
        dma_in(1)
        dma_in(2)
        emit_qk(0, 0, 4)
        alloc_sc(0, 0)
        alloc_sc(0, 1)
        emit_sc_group(0, 0)
        emit_sc_group(0, 1)

        # ---- main loop ----
        # PE emission order per iteration matches dependency-readiness so
        # the in-order PE queue never head-of-line blocks (see docstring).
        for t in range(T):
            emit_exp(t, 0)
            emit_exp(t, 1)
            if t >= 1:
                emit_av(t - 1, 2)
                emit_norm(t - 1)
            if t + 3 < T:
                dma_in(t + 3)
            if t + 1 < T:
                emit_qk(t + 1, 0, 2)
                ex[t + 1] = exp_pool.tile([128, 16, R], f16, tag="ex",
                                          name="ex")
            emit_sc_group(t, 2)
            emit_exp(t, 2)
            if t + 1 < T:
                emit_qk(t + 1, 2, 4)
            emit_av(t, 3)
            emit_sc_group(t, 3)
            emit_exp(t, 3)
            if t >= 1:
                emit_out_chunk(t - 1, 0)
            emit_av(t, 0)
            emit_sc_group(t, 4)
            emit_exp(t, 4)
            if t >= 1:
                emit_out_chunk(t - 1, 1)
            emit_sc_group(t, 5)
            emit_exp(t, 5)
            emit_av(t, 1)
            if t >= 1:
                emit_out_chunk(t - 1, 2)
            if t + 1 < T:
                alloc_sc(t + 1, 0)
                emit_sc_group(t + 1, 0)
            if t >= 1:
                emit_out_chunk(t - 1, 3)
            if t + 1 < T:
                alloc_sc(t + 1, 1)
                emit_sc_group(t + 1, 1)

        # ---- epilogue ----
        emit_av(T - 1, 2)
        emit_norm(T - 1)
        for c in range(4):
            emit_out_chunk(T - 1, c, pingpong=True)

        for pool in (ps_o, ps_av, ps_sc, outsb, workp, exp_pool, ctxp, io,
                     consts):
            pool.release()

    nc.compile()
    return nc


def _plan_supertiles(lengths):
    """Split each batch's valid rows into R-row supertiles; spread over cores."""
    tiles = []  # (batch, row0, nvalid)
    for b in range(B):
        nb = int(lengths[b])
        r0 = 0
        while r0 < nb:
            tiles.append((b, r0, min(R, nb - r0)))
            r0 += R
    T = max(1, (len(tiles) + N_CORES - 1) // N_CORES)
    per_core = [tiles[c * T: (c + 1) * T] for c in range(N_CORES)]
    return per_core, T


def kernel(xF, context, lengths, Wq, Wk, Wv, Wo, bo):
    from concourse import bass_utils

    xF = np.asarray(xF, np.float32)
    context = np.asarray(context, np.float32)
    lengths_np = np.asarray(lengths, np.int32)

    wqk, wvo = _build_host_constants(
        np.asarray(Wq, np.float32),
        np.asarray(Wk, np.float32),
        np.asarray(Wv, np.float32),
        np.asarray(Wo, np.float32),
        np.asarray(bo, np.float32),
    )
    ctx_b, c5_b = _build_context(context)

    per_core, T = _plan_supertiles(lengths_np)
    nc = _build_program(T)

    in_maps = []
    for c in range(N_CORES):
        xft = np.zeros((T, 128, 4, R), np.float16)
        ctx = np.zeros((T, 128, 2, 256), np.float16)
        c5 = np.zeros((T, 128, 2, 64), np.float16)
        c5[:, :, :, 4] = 1.0  # dummy tiles: finite normalizer
        c5[:, :, :, 32 + 20] = 1.0
        for t, (b, r0, nv) in enumerate(per_core[c]):
            blockT = np.zeros((CH, R), np.float32)
            blockT[:, :nv] = xF[b, r0: r0 + nv, :].T
            xft[t] = blockT.reshape(4, 128, R).transpose(1, 0, 2)
            ctx[t] = ctx_b[b]
            c5[t] = c5_b[b]
        in_maps.append(
            {
                "xft": xft,
                "ctx": ctx,
                "c5": c5,
                "wqk": wqk,
                "wvo": wvo,
            }
        )

    import os

    trace = bool(os.environ.get("CA_TRACE"))
    res = bass_utils.run_bass_kernel_spmd(
        nc,
        in_maps,
        core_ids=list(range(N_CORES)),
        trace=trace,
        **({"tmpdir": "/tmp/ca_prof"} if trace else {}),
    )
    if trace and res.exec_time_ns is not None:
        print(f"HW exec time: {res.exec_time_ns} ns")

    out = np.zeros((B, L, CH), np.float32)
    for c in range(N_CORES):
        arr = np.asarray(res.results[c]["outt"], np.float32)  # [T, 128, 4, R]
        for t, (b, r0, nv) in enumerate(per_core[c]):
            rows = arr[t].transpose(2, 1, 0).reshape(R, CH)  # [row, ch]
            out[b, r0: r0 + nv, :] = rows[:nv]
    return out


# revision 38
# speedup vs baseline: 1.1093x; 1.1093x over previous
"""Trainium2 Bass kernel for nn_CrossAttention (sparse cross-attention).

Math (reference):
    q = xF @ Wq;  k = context @ Wk;  v = context @ Wv
    attn = softmax(scale * q k^T) over K keys
    out = (attn v) @ Wo + bo + xF, rows >= lengths[b] zeroed

Algebraic restructure (context has only 4 channels, so per-head k/v are
rank-4):
    Wqk[ch, h, e] = scale * sum_d Wq[ch, hD+d] Wk[e, hD+d]
    sT_h[key,row] = ctx[key,:] . (Wqk_h^T xF_row)         (PE, contraction 4)
    avT[(h,e), row] = sum_k exp(sT_h) [ctx;1][k,e]        (PE, contraction 128)
    out = Wvo^T (avT / lam) + bo + xF,   Wvo_h = Wv_h Wo_h

Single-pass interleaved head layout: all 8 heads live in one 128-partition
tile.  qk components at partitions 32g+4p+e (head 2g+p); av/avn at
partitions 16h+e (e<4: value comps, e=4: softmax normalizer from a ones
column).  Score matmuls are 4-way row-banded (strip g), av matmuls 4-way
col-banded; banded matmuls at disjoint tile positions execute concurrently
on the PE array, so emission batches them band-rotated.

Engine balance per supertile (R=512 rows, measured on hw): the Act-engine
exp (16 strips x 512 cols, ~8.4us with overheads) and the PE stream are
co-critical.  The PE executes ~540ns batch slots in which matmuls at
disjoint 32-wide tile positions run concurrently, so work is batched for
maximal in-slot concurrency: scores in 6 groups of [3,3,3,3,2,2] strips
(bounded by PSUM: 2 ping-pong slots x 3 banks), av in 4 quads of 4
col-bands (accumulation flags span the quads), qk as two 2-chunk
accumulation runs, out as 4 spread chunks sharing the qk PSUM bank
(WAR-paced by the DVE residual adds).  The softmax normalization is an
all-DVE chain reading av straight from PSUM (stream_shuffle of the
ones-row, fast reciprocal, multiply) so avn is ready early and gpsimd
stays free for DMA posting.  Inputs DMA 3 tiles ahead (xft on the sync
queue ring, ctx/c5 on the gpsimd ring, one descriptor set per tensor --
finer splits choke the ring on descriptor rate); output leaves per
128-channel chunk on the gpsimd ring.  Emission order is a priority hint
to the tile list-scheduler; this ordering is a tuned local optimum
(reorderings and priority boosts measured worse).
"""

import numpy as np

NUM_HEAD = 8
CH_HEAD = 64
CH = 512
CONTEXT_CH = 4
B, L, K = 16, 4096, 256
R = 512
N_CORES = 8

# exp groups: (slot 'A'|'B', strip start, strip end)
GROUPS = [("A", 0, 3), ("B", 3, 6), ("A", 6, 9), ("B", 9, 12),
          ("A", 12, 14), ("B", 14, 16)]


def _strip_info(s):
    g, wave = s % 4, s // 4
    return g, wave // 2, wave % 2  # row/col strip, parity, key chunk


def _build_host_constants(Wq, Wk, Wv, Wo, bo):
    scale = CH_HEAD ** (-0.5)
    Wq_h = Wq.reshape(CH, NUM_HEAD, CH_HEAD)
    Wk_h = Wk.reshape(CONTEXT_CH, NUM_HEAD, CH_HEAD)
    Wqk = scale * np.einsum("chd,ehd->che", Wq_h, Wk_h)  # [512, 8, 4]

    wqk = np.zeros((128, 4, 128), np.float16)  # [ch_part, chunk, m]
    for g in range(4):
        for par in range(2):
            h = 2 * g + par
            for c in range(4):
                wqk[:, c, 32 * g + 4 * par: 32 * g + 4 * par + 4] = \
                    Wqk[128 * c: 128 * (c + 1), h, :]

    Wv_h = Wv.reshape(CONTEXT_CH, NUM_HEAD, CH_HEAD)
    Wo_h = Wo.reshape(NUM_HEAD, CH_HEAD, CH)
    wvo = np.zeros((128, CH), np.float16)
    for h in range(NUM_HEAD):
        wvo[16 * h: 16 * h + 4, :] = Wv_h[:, h, :] @ Wo_h[h]
        wvo[16 * h + 4, :] = bo / NUM_HEAD  # avn row 16h+4 == 1.0 exactly
    return wqk, wvo


def _build_context(context):
    ctx = np.zeros((B, 128, 2, 256), np.float16)   # [...][:128]=A, [128:]=B
    c5 = np.zeros((B, 128, 2, 64), np.float16)     # [...][:32]=A, [32:]=B
    for b in range(B):
        cT = context[b].T  # [4, 256]
        for g in range(4):
            for kc in range(2):
                ctx[b, 32 * g: 32 * g + 4, kc, 0:128] = cT[:, 128 * kc:128 * (kc + 1)]
                ctx[b, 32 * g + 4: 32 * g + 8, kc, 128:256] = cT[:, 128 * kc:128 * (kc + 1)]
        for kc in range(2):
            c5[b, :, kc, 0:4] = context[b, 128 * kc:128 * (kc + 1), :]
            c5[b, :, kc, 4] = 1.0
            c5[b, :, kc, 32 + 16:32 + 20] = context[b, 128 * kc:128 * (kc + 1), :]
            c5[b, :, kc, 32 + 20] = 1.0
    return ctx, c5


def _build_program(T):
    import concourse.bass as bass  # noqa: F401
    import concourse.tile as tile
    from concourse import bacc, mybir

    f32 = mybir.dt.float32
    f16 = mybir.dt.float16
    Exp = mybir.ActivationFunctionType.Exp

    nc = bacc.Bacc("TRN2", target_bir_lowering=False, debug=False)

    xft_d = nc.dram_tensor("xft", [T, 128, 4, R], f16, kind="ExternalInput").ap()
    ctx_d = nc.dram_tensor("ctx", [T, 128, 2, 256], f16, kind="ExternalInput").ap()
    c5_d = nc.dram_tensor("c5", [T, 128, 2, 64], f16, kind="ExternalInput").ap()
    wqk_d = nc.dram_tensor("wqk", [128, 4, 128], f16, kind="ExternalInput").ap()
    wvo_d = nc.dram_tensor("wvo", [128, CH], f16, kind="ExternalInput").ap()
    out_d = nc.dram_tensor("outt", [T, 128, 4, R], f16, kind="ExternalOutput").ap()

    with tile.TileContext(nc) as tc:
        consts = tc.alloc_tile_pool(name="consts", bufs=1)
        wqk_s = consts.tile([128, 4, 128], f16)
        wvo_s = consts.tile([128, CH], f16)
        expbias = consts.tile([128, 1], f32)
        nc.vector.memset(expbias, -4.0)
        nc.scalar.dma_start(out=wqk_s, in_=wqk_d)
        nc.scalar.dma_start(out=wvo_s, in_=wvo_d)
        # Warm up the Act engine's Exp table during the DMA fill window;
        # otherwise the first real exp pays the ~1.3us ACT_TABLE_LOAD inline.
        actwarm = consts.tile([128, 1], f32)
        nc.scalar.activation(out=actwarm, in_=expbias, func=Exp)

        io = tc.alloc_tile_pool(name="io", bufs=5)
        ctxp = tc.alloc_tile_pool(name="ctxp", bufs=4)
        exp_pool = tc.alloc_tile_pool(name="exp_pool", bufs=2)
        workp = tc.alloc_tile_pool(name="workp", bufs=2)
        outsb = tc.alloc_tile_pool(name="outsb", bufs=2)
        ps_sc = tc.alloc_tile_pool(name="ps_sc", bufs=1, space="PSUM")
        ps_av = tc.alloc_tile_pool(name="ps_av", bufs=1, space="PSUM")
        ps_o = tc.alloc_tile_pool(name="ps_o", bufs=1, space="PSUM")

        # per-iteration live state
        xft = [None] * T
        ctxt = [None] * T
        c5t = [None] * T
        qk_sb = [None] * T
        ex = [None] * T
        av_ps = [None] * T
        avn = [None] * T
        out_sb = [None] * T
        sc_tiles = {}  # (t, group) -> psum tile

        def dma_in(t):
            # xft on the sync queue; ctx/c5 on the gpsimd queue
            xft[t] = io.tile([128, 4, R], f16, tag="xft", name="xft")
            nc.sync.dma_start(out=xft[t], in_=xft_d[t])
            ctxt[t] = ctxp.tile([128, 2, 256], f16, tag="ctx", name="ctx")
            nc.gpsimd.dma_start(out=ctxt[t], in_=ctx_d[t])
            c5t[t] = ctxp.tile([128, 2, 64], f16, tag="c5", name="c5")
            nc.gpsimd.dma_start(out=c5t[t], in_=c5_d[t])

        def alloc_sc(t, k):
            slot = GROUPS[k][0]
            sc_tiles[(t, k)] = ps_sc.tile([128, 3, R], f32, tag=f"sc{slot}",
                                          name=f"sc{slot}")
            return sc_tiles[(t, k)]

        def emit_sc_group(t, k):
            """Score matmuls for exp group k of tile t (row-banded)."""
            sc = sc_tiles.get((t, k))
            if sc is None:
                sc = alloc_sc(t, k)
            _, s0, s1 = GROUPS[k]
            for i, s in enumerate(range(s0, s1)):
                g, par, kc = _strip_info(s)
                nc.tensor.matmul(
                    out=sc[:, i, :],
                    lhsT=ctxt[t][32 * g: 32 * g + 8, kc,
                                 128 * par: 128 * (par + 1)],
                    rhs=qk_sb[t][32 * g: 32 * g + 8, :],
                    tile_position=(32 * g, 0),
                )

        def emit_exp(t, k):
            if t == 0 and k == 0:
                ex[t] = exp_pool.tile([128, 16, R], f16, tag="ex", name="ex")
            _, s0, s1 = GROUPS[k]
            n = s1 - s0
            nc.scalar.activation(
                out=ex[t][:, s0:s1, :],
                in_=sc_tiles[(t, k)][:, 0:n, :],
                func=Exp,
                bias=expbias,
            )

        qk_ps = [None] * T

        def emit_qk(t, c0, c1):
            """qk projection chunks [c0,c1) for tile t (shared out/qk bank)."""
            if c0 == 0:
                qk_ps[t] = ps_o.tile([128, R], f32, tag="o", name="qk_ps")
            for c in range(c0, c1):
                nc.tensor.matmul(
                    out=qk_ps[t],
                    lhsT=wqk_s[:, c, :],
                    rhs=xft[t][:, c, :],
                    start=(c == 0),
                    stop=(c == 3),
                )
            if c1 == 4:
                qk_sb[t] = workp.tile([128, R], f16, tag="qk_sb", name="qk_sb")
                nc.vector.tensor_copy(out=qk_sb[t], in_=qk_ps[t])

        def emit_av(t, w):
            """av matmuls for strip quad w (4 col-banded, concurrent)."""
            if w == 0:
                av_ps[t] = ps_av.tile([128, R], f32, tag="avlam", name="av_ps")
            for s in range(4 * w, 4 * w + 4):
                g, par, kc = _strip_info(s)
                nc.tensor.matmul(
                    out=av_ps[t][32 * g: 32 * (g + 1), :],
                    lhsT=c5t[t][:, kc, 32 * par: 32 * par + 32],
                    rhs=ex[t][:, s, :],
                    tile_position=(0, 32 * g),
                    start=(s < 4),
                    stop=(s >= 12),
                )

        def emit_norm(t):
            # all-DVE, av read straight from PSUM (shortest avn latency)
            lam_sb = workp.tile([128, R], f32, tag="lam_sb")
            avn[t] = workp.tile([128, R], f16, tag="avn", name="avn")
            lr_sb = workp.tile([128, R], f32, tag="lr_sb")
            nc.vector.stream_shuffle(out=lam_sb, in_=av_ps[t],
                                     mask=[4] * 16 + [20] * 16)
            nc.vector.reciprocal_approx_fast(out=lr_sb, in_=lam_sb)
            nc.vector.tensor_mul(avn[t], av_ps[t], lr_sb)

        def emit_out_chunk(t, c, pingpong=False, borrow=None):
            """Out-projection chunk c of tile t.

            pingpong (epilogue only): odd chunks use the freed av bank so
            the tail out chain isn't WAR-paced on a single PSUM bank.
            borrow=(tile, group): steady state, write into the idle 3rd
            bank of a 2-strip score group's slot instead of the shared o
            bank, breaking the o-bank WAR chain between consecutive chunks.
            """
            if c == 0:
                out_sb[t] = outsb.tile([128, 4, R], f16, tag="out_sb",
                                       name="out_sb")
            if borrow is not None:
                o_ps = sc_tiles[borrow][:, 2, :]
            elif pingpong and c % 2:
                o_ps = ps_av.tile([128, R], f32, tag="avlam", name="o2_ps")
            else:
                o_ps = ps_o.tile([128, R], f32, tag="o", name="o_ps")
            nc.tensor.matmul(
                out=o_ps,
                lhsT=wvo_s[:, 128 * c: 128 * (c + 1)],
                rhs=avn[t],
            )
            nc.vector.tensor_add(out_sb[t][:, c, :], o_ps, xft[t][:, c, :])
            nc.gpsimd.dma_start(out=out_d[t][:, c, :], in_=out_sb[t][:, c, :])

        # ---- prologue ----
        dma_in(0)
        dma_in(1)
        dma_in(2)
        emit_qk(0, 0, 4)
        alloc_sc(0, 0)
        alloc_sc(0, 1)
        emit_sc_group(0, 0)
        emit_sc_group(0, 1)

        # ---- main loop ----
        # PE emission order per iteration matches dependency-readiness so
        # the in-order PE queue never head-of-line blocks (see docstring).
        for t in range(T):
            emit_exp(t, 0)
            emit_exp(t, 1)
            if t >= 1:
                emit_av(t - 1, 3)
                emit_norm(t - 1)
            if t + 3 < T:
                dma_in(t + 3)
            if t + 1 < T:
                emit_qk(t + 1, 0, 2)
                ex[t + 1] = exp_pool.tile([128, 16, R], f16, tag="ex",
                                          name="ex")
            emit_sc_group(t, 2)
            emit_exp(t, 2)
            if t + 1 < T:
                emit_qk(t + 1, 2, 4)
            emit_av(t, 0)
            emit_sc_group(t, 3)
            emit_exp(t, 3)
            if t >= 1:
                emit_out_chunk(t - 1, 0)
            emit_av(t, 1)
            emit_sc_group(t, 4)
            emit_exp(t, 4)
            if t >= 1:
                emit_out_chunk(t - 1, 1)
            emit_sc_group(t, 5)
            emit_exp(t, 5)
            emit_av(t, 2)
            if t >= 1:
                emit_out_chunk(t - 1, 2)
            if t + 1 < T:
                alloc_sc(t + 1, 0)
                emit_sc_group(t + 1, 0)
            if t >= 1:
                emit_out_chunk(t - 1, 3)
            if t + 1 < T:
                alloc_sc(t + 1, 1)
                emit_sc_group(t + 1, 1)

        # ---- epilogue ----
        emit_av(T - 1, 3)
        emit_norm(T - 1)
        for c in range(4):
            emit_out_chunk(T - 1, c, pingpong=True)

        for pool in (ps_o, ps_av, ps_sc, outsb, workp, exp_pool, ctxp, io,
                     consts):
            pool.release()

    nc.compile()
    return nc


def _plan_supertiles(lengths):
    """Split each batch's valid rows into R-row supertiles; spread over cores."""
    tiles = []  # (batch, row0, nvalid)
    for b in range(B):
        nb = int(lengths[b])
        r0 = 0
        while r0 < nb:
            tiles.append((b, r0, min(R, nb - r0)))
            r0 += R
    T = max(1, (len(tiles) + N_CORES - 1) // N_CORES)
    per_core = [tiles[c * T: (c + 1) * T] for c in range(N_CORES)]
    return per_core, T


def kernel(xF, context, lengths, Wq, Wk, Wv, Wo, bo):
    from concourse import bass_utils

    xF = np.asarray(xF, np.float32)
    context = np.asarray(context, np.float32)
    lengths_np = np.asarray(lengths, np.int32)

    wqk, wvo = _build_host_constants(
        np.asarray(Wq, np.float32),
        np.asarray(Wk, np.float32),
        np.asarray(Wv, np.float32),
        np.asarray(Wo, np.float32),
        np.asarray(bo, np.float32),
    )
    ctx_b, c5_b = _build_context(context)

    per_core, T = _plan_supertiles(lengths_np)
    nc = _build_program(T)

    in_maps = []
    for c in range(N_CORES):
        xft = np.zeros((T, 128, 4, R), np.float16)
        ctx = np.zeros((T, 128, 2, 256), np.float16)
        c5 = np.zeros((T, 128, 2, 64), np.float16)
        c5[:, :, :, 4] = 1.0  # dummy tiles: finite normalizer
        c5[:, :, :, 32 + 20] = 1.0
        for t, (b, r0, nv) in enumerate(per_core[c]):
            blockT = np.zeros((CH, R), np.float32)
            blockT[:, :nv] = xF[b, r0: r0 + nv, :].T
            xft[t] = blockT.reshape(4, 128, R).transpose(1, 0, 2)
            ctx[t] = ctx_b[b]
            c5[t] = c5_b[b]
        in_maps.append(
            {
                "xft": xft,
                "ctx": ctx,
                "c5": c5,
                "wqk": wqk,
                "wvo": wvo,
            }
        )

    import os

    trace = bool(os.environ.get("CA_TRACE"))
    res = bass_utils.run_bass_kernel_spmd(
        nc,
        in_maps,
        core_ids=list(range(N_CORES)),
        trace=trace,
        **({"tmpdir": "/tmp/ca_prof"} if trace else {}),
    )
    if trace and res.exec_time_ns is not None:
        print(f"HW exec time: {res.exec_time_ns} ns")

    out = np.zeros((B, L, CH), np.float32)
    for c in range(N_CORES):
        arr = np.asarray(res.results[c]["outt"], np.float32)  # [T, 128, 4, R]
        for t, (b, r0, nv) in enumerate(per_core[c]):
            rows = arr[t].transpose(2, 1, 0).reshape(R, CH)  # [row, ch]
            out[b, r0: r0 + nv, :] = rows[:nv]
    return out


# revision 39
# speedup vs baseline: 1.1233x; 1.0127x over previous
"""Trainium2 Bass kernel for nn_CrossAttention (sparse cross-attention).

Math (reference):
    q = xF @ Wq;  k = context @ Wk;  v = context @ Wv
    attn = softmax(scale * q k^T) over K keys
    out = (attn v) @ Wo + bo + xF, rows >= lengths[b] zeroed

Algebraic restructure (context has only 4 channels, so per-head k/v are
rank-4):
    Wqk[ch, h, e] = scale * sum_d Wq[ch, hD+d] Wk[e, hD+d]
    sT_h[key,row] = ctx[key,:] . (Wqk_h^T xF_row)         (PE, contraction 4)
    avT[(h,e), row] = sum_k exp(sT_h) [ctx;1][k,e]        (PE, contraction 128)
    out = Wvo^T (avT / lam) + bo + xF,   Wvo_h = Wv_h Wo_h

Single-pass interleaved head layout: all 8 heads live in one 128-partition
tile.  qk components at partitions 32g+4p+e (head 2g+p); av/avn at
partitions 16h+e (e<4: value comps, e=4: softmax normalizer from a ones
column).  Score matmuls are 4-way row-banded (strip g), av matmuls 4-way
col-banded; banded matmuls at disjoint tile positions execute concurrently
on the PE array, so emission batches them band-rotated.

Engine balance per supertile (R=512 rows, measured on hw): the Act-engine
exp (16 strips x 512 cols, ~8.4us with overheads) and the PE stream are
co-critical.  The PE executes ~540ns batch slots in which matmuls at
disjoint 32-wide tile positions run concurrently, so work is batched for
maximal in-slot concurrency: scores in 6 groups of [3,3,3,3,2,2] strips
(bounded by PSUM: 2 ping-pong slots x 3 banks), av in 4 quads of 4
col-bands (accumulation flags span the quads), qk as two 2-chunk
accumulation runs, out as 4 spread chunks sharing the qk PSUM bank
(WAR-paced by the DVE residual adds).  The softmax normalization is an
all-DVE chain reading av straight from PSUM (stream_shuffle of the
ones-row, fast reciprocal, multiply) so avn is ready early and gpsimd
stays free for DMA posting.  Inputs DMA 3 tiles ahead (xft on the sync
queue ring, ctx/c5 on the gpsimd ring, one descriptor set per tensor --
finer splits choke the ring on descriptor rate); output leaves per
128-channel chunk on the gpsimd ring.  Emission order is a priority hint
to the tile list-scheduler; this ordering is a tuned local optimum
(reorderings and priority boosts measured worse).
"""

import numpy as np

NUM_HEAD = 8
CH_HEAD = 64
CH = 512
CONTEXT_CH = 4
B, L, K = 16, 4096, 256
R = 512
N_CORES = 8

# exp groups: (slot 'A'|'B', strip start, strip end)
GROUPS = [("A", 0, 3), ("B", 3, 6), ("A", 6, 9), ("B", 9, 12),
          ("A", 12, 14), ("B", 14, 16)]


def _strip_info(s):
    g, wave = s % 4, s // 4
    return g, wave // 2, wave % 2  # row/col strip, parity, key chunk


def _build_host_constants(Wq, Wk, Wv, Wo, bo):
    scale = CH_HEAD ** (-0.5)
    Wq_h = Wq.reshape(CH, NUM_HEAD, CH_HEAD)
    Wk_h = Wk.reshape(CONTEXT_CH, NUM_HEAD, CH_HEAD)
    Wqk = scale * np.einsum("chd,ehd->che", Wq_h, Wk_h)  # [512, 8, 4]

    wqk = np.zeros((128, 4, 128), np.float16)  # [ch_part, chunk, m]
    for g in range(4):
        for par in range(2):
            h = 2 * g + par
            for c in range(4):
                wqk[:, c, 32 * g + 4 * par: 32 * g + 4 * par + 4] = \
                    Wqk[128 * c: 128 * (c + 1), h, :]

    Wv_h = Wv.reshape(CONTEXT_CH, NUM_HEAD, CH_HEAD)
    Wo_h = Wo.reshape(NUM_HEAD, CH_HEAD, CH)
    wvo = np.zeros((128, CH), np.float16)
    for h in range(NUM_HEAD):
        wvo[16 * h: 16 * h + 4, :] = Wv_h[:, h, :] @ Wo_h[h]
        wvo[16 * h + 4, :] = bo / NUM_HEAD  # avn row 16h+4 == 1.0 exactly
    return wqk, wvo


def _build_context(context):
    ctx = np.zeros((B, 128, 2, 256), np.float16)   # [...][:128]=A, [128:]=B
    c5 = np.zeros((B, 128, 2, 64), np.float16)     # [...][:32]=A, [32:]=B
    for b in range(B):
        cT = context[b].T  # [4, 256]
        for g in range(4):
            for kc in range(2):
                ctx[b, 32 * g: 32 * g + 4, kc, 0:128] = cT[:, 128 * kc:128 * (kc + 1)]
                ctx[b, 32 * g + 4: 32 * g + 8, kc, 128:256] = cT[:, 128 * kc:128 * (kc + 1)]
        for kc in range(2):
            c5[b, :, kc, 0:4] = context[b, 128 * kc:128 * (kc + 1), :]
            c5[b, :, kc, 4] = 1.0
            c5[b, :, kc, 32 + 16:32 + 20] = context[b, 128 * kc:128 * (kc + 1), :]
            c5[b, :, kc, 32 + 20] = 1.0
    return ctx, c5


def _build_program(T, half_last):
    import concourse.bass as bass  # noqa: F401
    import concourse.tile as tile
    from concourse import bacc, mybir

    f32 = mybir.dt.float32
    f16 = mybir.dt.float16
    Exp = mybir.ActivationFunctionType.Exp

    nc = bacc.Bacc("TRN2", target_bir_lowering=False, debug=False)

    xft_d = nc.dram_tensor("xft", [T, 128, 4, R], f16, kind="ExternalInput").ap()
    ctx_d = nc.dram_tensor("ctx", [T, 128, 2, 256], f16, kind="ExternalInput").ap()
    c5_d = nc.dram_tensor("c5", [T, 128, 2, 64], f16, kind="ExternalInput").ap()
    wqk_d = nc.dram_tensor("wqk", [128, 4, 128], f16, kind="ExternalInput").ap()
    wvo_d = nc.dram_tensor("wvo", [128, CH], f16, kind="ExternalInput").ap()
    out_d = nc.dram_tensor("outt", [T, 128, 4, R], f16, kind="ExternalOutput").ap()

    with tile.TileContext(nc) as tc:
        consts = tc.alloc_tile_pool(name="consts", bufs=1)
        wqk_s = consts.tile([128, 4, 128], f16)
        wvo_s = consts.tile([128, CH], f16)
        expbias = consts.tile([128, 1], f32)
        nc.vector.memset(expbias, -4.0)
        nc.scalar.dma_start(out=wqk_s, in_=wqk_d)
        nc.scalar.dma_start(out=wvo_s, in_=wvo_d)
        # Warm up the Act engine's Exp table during the DMA fill window;
        # otherwise the first real exp pays the ~1.3us ACT_TABLE_LOAD inline.
        actwarm = consts.tile([128, 1], f32)
        nc.scalar.activation(out=actwarm, in_=expbias, func=Exp)

        io = tc.alloc_tile_pool(name="io", bufs=5)
        ctxp = tc.alloc_tile_pool(name="ctxp", bufs=4)
        exp_pool = tc.alloc_tile_pool(name="exp_pool", bufs=2)
        workp = tc.alloc_tile_pool(name="workp", bufs=2)
        outsb = tc.alloc_tile_pool(name="outsb", bufs=2)
        ps_sc = tc.alloc_tile_pool(name="ps_sc", bufs=1, space="PSUM")
        ps_av = tc.alloc_tile_pool(name="ps_av", bufs=1, space="PSUM")
        ps_o = tc.alloc_tile_pool(name="ps_o", bufs=1, space="PSUM")

        # free-dim (row) width per tile: the last tile holds only small
        # batch tails (<=R/2 valid rows) and runs at half width
        W = [R] * T
        if half_last:
            W[T - 1] = R // 2

        # per-iteration live state
        xft = [None] * T
        ctxt = [None] * T
        c5t = [None] * T
        qk_sb = [None] * T
        ex = [None] * T
        av_ps = [None] * T
        avn = [None] * T
        out_sb = [None] * T
        sc_tiles = {}  # (t, group) -> psum tile

        def dma_in(t):
            # xft on the sync queue; ctx/c5 on the gpsimd queue
            xft[t] = io.tile([128, 4, R], f16, tag="xft", name="xft")
            nc.sync.dma_start(out=xft[t], in_=xft_d[t])
            ctxt[t] = ctxp.tile([128, 2, 256], f16, tag="ctx", name="ctx")
            nc.gpsimd.dma_start(out=ctxt[t], in_=ctx_d[t])
            c5t[t] = ctxp.tile([128, 2, 64], f16, tag="c5", name="c5")
            nc.gpsimd.dma_start(out=c5t[t], in_=c5_d[t])

        def alloc_sc(t, k):
            slot = GROUPS[k][0]
            sc_tiles[(t, k)] = ps_sc.tile([128, 3, R], f32, tag=f"sc{slot}",
                                          name=f"sc{slot}")
            return sc_tiles[(t, k)]

        def emit_sc_group(t, k):
            """Score matmuls for exp group k of tile t (row-banded)."""
            sc = sc_tiles.get((t, k))
            if sc is None:
                sc = alloc_sc(t, k)
            _, s0, s1 = GROUPS[k]
            for i, s in enumerate(range(s0, s1)):
                g, par, kc = _strip_info(s)
                nc.tensor.matmul(
                    out=sc[:, i, : W[t]],
                    lhsT=ctxt[t][32 * g: 32 * g + 8, kc,
                                 128 * par: 128 * (par + 1)],
                    rhs=qk_sb[t][32 * g: 32 * g + 8, : W[t]],
                    tile_position=(32 * g, 0),
                )

        def emit_exp(t, k):
            if t == 0 and k == 0:
                ex[t] = exp_pool.tile([128, 16, R], f16, tag="ex", name="ex")
            _, s0, s1 = GROUPS[k]
            n = s1 - s0
            nc.scalar.activation(
                out=ex[t][:, s0:s1, : W[t]],
                in_=sc_tiles[(t, k)][:, 0:n, : W[t]],
                func=Exp,
                bias=expbias,
            )

        qk_ps = [None] * T

        def emit_qk(t, c0, c1):
            """qk projection chunks [c0,c1) for tile t (shared out/qk bank)."""
            if c0 == 0:
                qk_ps[t] = ps_o.tile([128, R], f32, tag="o", name="qk_ps")
            for c in range(c0, c1):
                nc.tensor.matmul(
                    out=qk_ps[t][:, : W[t]],
                    lhsT=wqk_s[:, c, :],
                    rhs=xft[t][:, c, : W[t]],
                    start=(c == 0),
                    stop=(c == 3),
                )
            if c1 == 4:
                qk_sb[t] = workp.tile([128, R], f16, tag="qk_sb", name="qk_sb")
                nc.vector.tensor_copy(out=qk_sb[t][:, : W[t]],
                                      in_=qk_ps[t][:, : W[t]])

        def emit_av(t, w):
            """av matmuls for strip quad w (4 col-banded, concurrent)."""
            if w == 0:
                av_ps[t] = ps_av.tile([128, R], f32, tag="avlam", name="av_ps")
            for s in range(4 * w, 4 * w + 4):
                g, par, kc = _strip_info(s)
                nc.tensor.matmul(
                    out=av_ps[t][32 * g: 32 * (g + 1), : W[t]],
                    lhsT=c5t[t][:, kc, 32 * par: 32 * par + 32],
                    rhs=ex[t][:, s, : W[t]],
                    tile_position=(0, 32 * g),
                    start=(s < 4),
                    stop=(s >= 12),
                )

        def emit_norm(t):
            # all-DVE, av read straight from PSUM (shortest avn latency)
            lam_sb = workp.tile([128, R], f32, tag="lam_sb")
            avn[t] = workp.tile([128, R], f16, tag="avn", name="avn")
            lr_sb = workp.tile([128, R], f32, tag="lr_sb")
            w = W[t]
            nc.vector.stream_shuffle(out=lam_sb[:, :w], in_=av_ps[t][:, :w],
                                     mask=[4] * 16 + [20] * 16)
            nc.vector.reciprocal_approx_fast(out=lr_sb[:, :w],
                                             in_=lam_sb[:, :w])
            nc.vector.tensor_mul(avn[t][:, :w], av_ps[t][:, :w],
                                 lr_sb[:, :w])

        def emit_out_chunk(t, c, pingpong=False, borrow=None):
            """Out-projection chunk c of tile t.

            pingpong (epilogue only): odd chunks use the freed av bank so
            the tail out chain isn't WAR-paced on a single PSUM bank.
            borrow=(tile, group): steady state, write into the idle 3rd
            bank of a 2-strip score group's slot instead of the shared o
            bank, breaking the o-bank WAR chain between consecutive chunks.
            """
            if c == 0:
                out_sb[t] = outsb.tile([128, 4, R], f16, tag="out_sb",
                                       name="out_sb")
            if borrow is not None:
                o_ps = sc_tiles[borrow][:, 2, :]
            elif pingpong and c % 2:
                o_ps = ps_av.tile([128, R], f32, tag="avlam", name="o2_ps")
            else:
                o_ps = ps_o.tile([128, R], f32, tag="o", name="o_ps")
            w = W[t]
            nc.tensor.matmul(
                out=o_ps[:, :w],
                lhsT=wvo_s[:, 128 * c: 128 * (c + 1)],
                rhs=avn[t][:, :w],
            )
            nc.vector.tensor_add(out_sb[t][:, c, :w], o_ps[:, :w],
                                 xft[t][:, c, :w])
            nc.gpsimd.dma_start(out=out_d[t][:, c, :w],
                                in_=out_sb[t][:, c, :w])

        # ---- prologue ----
        dma_in(0)
        dma_in(1)
        dma_in(2)
        emit_qk(0, 0, 4)
        alloc_sc(0, 0)
        alloc_sc(0, 1)
        emit_sc_group(0, 0)
        emit_sc_group(0, 1)

        # ---- main loop ----
        # PE emission order per iteration matches dependency-readiness so
        # the in-order PE queue never head-of-line blocks (see docstring).
        for t in range(T):
            emit_exp(t, 0)
            emit_exp(t, 1)
            if t >= 1:
                emit_av(t - 1, 3)
                emit_norm(t - 1)
            if t + 3 < T:
                dma_in(t + 3)
            if t + 1 < T:
                emit_qk(t + 1, 0, 2)
                ex[t + 1] = exp_pool.tile([128, 16, R], f16, tag="ex",
                                          name="ex")
            emit_sc_group(t, 2)
            emit_exp(t, 2)
            if t + 1 < T:
                emit_qk(t + 1, 2, 4)
            emit_av(t, 0)
            emit_sc_group(t, 3)
            emit_exp(t, 3)
            if t >= 1:
                emit_out_chunk(t - 1, 0)
            emit_av(t, 1)
            emit_sc_group(t, 4)
            emit_exp(t, 4)
            if t >= 1:
                emit_out_chunk(t - 1, 1)
            emit_sc_group(t, 5)
            emit_exp(t, 5)
            emit_av(t, 2)
            if t >= 1:
                emit_out_chunk(t - 1, 2)
            if t + 1 < T:
                alloc_sc(t + 1, 0)
                emit_sc_group(t + 1, 0)
            if t >= 1:
                emit_out_chunk(t - 1, 3)
            if t + 1 < T:
                alloc_sc(t + 1, 1)
                emit_sc_group(t + 1, 1)

        # ---- epilogue ----
        emit_av(T - 1, 3)
        emit_norm(T - 1)
        for c in range(4):
            emit_out_chunk(T - 1, c, pingpong=True)

        for pool in (ps_o, ps_av, ps_sc, outsb, workp, exp_pool, ctxp, io,
                     consts):
            pool.release()

    nc.compile()
    return nc


def _plan_supertiles(lengths):
    """Split valid rows into R-row supertiles, spread over cores.

    Tiles with <= R/2 valid rows (small batch tails) are packed into one
    half-width slot at the end of each core, so the last program tile runs
    at half cost.  Falls back to all-full tiles when the split doesn't fit
    one small tile per core."""
    tiles = []  # (batch, row0, nvalid)
    for b in range(B):
        nb = int(lengths[b])
        r0 = 0
        while r0 < nb:
            tiles.append((b, r0, min(R, nb - r0)))
            r0 += R
    big = [x for x in tiles if x[2] > R // 2]
    small = [x for x in tiles if x[2] <= R // 2]
    tb = (len(big) + N_CORES - 1) // N_CORES
    half_last = 0 < len(small) <= N_CORES and tb * N_CORES >= len(big)
    if not half_last:
        T = max(1, (len(tiles) + N_CORES - 1) // N_CORES)
        return [tiles[c * T: (c + 1) * T] for c in range(N_CORES)], T, False
    per_core = []
    for c in range(N_CORES):
        lst = big[c * tb: (c + 1) * tb]
        lst += [None] * (tb - len(lst))
        lst.append(small[c] if c < len(small) else None)
        per_core.append(lst)
    return per_core, tb + 1, True


def kernel(xF, context, lengths, Wq, Wk, Wv, Wo, bo):
    from concourse import bass_utils

    xF = np.asarray(xF, np.float32)
    context = np.asarray(context, np.float32)
    lengths_np = np.asarray(lengths, np.int32)

    wqk, wvo = _build_host_constants(
        np.asarray(Wq, np.float32),
        np.asarray(Wk, np.float32),
        np.asarray(Wv, np.float32),
        np.asarray(Wo, np.float32),
        np.asarray(bo, np.float32),
    )
    ctx_b, c5_b = _build_context(context)

    per_core, T, half_last = _plan_supertiles(lengths_np)
    nc = _build_program(T, half_last)

    in_maps = []
    for c in range(N_CORES):
        xft = np.zeros((T, 128, 4, R), np.float16)
        ctx = np.zeros((T, 128, 2, 256), np.float16)
        c5 = np.zeros((T, 128, 2, 64), np.float16)
        c5[:, :, :, 4] = 1.0  # dummy tiles: finite normalizer
        c5[:, :, :, 32 + 20] = 1.0
        for t, seg in enumerate(per_core[c]):
            if seg is None:
                continue
            b, r0, nv = seg
            blockT = np.zeros((CH, R), np.float32)
            blockT[:, :nv] = xF[b, r0: r0 + nv, :].T
            xft[t] = blockT.reshape(4, 128, R).transpose(1, 0, 2)
            ctx[t] = ctx_b[b]
            c5[t] = c5_b[b]
        in_maps.append(
            {
                "xft": xft,
                "ctx": ctx,
                "c5": c5,
                "wqk": wqk,
                "wvo": wvo,
            }
        )

    import os

    trace = bool(os.environ.get("CA_TRACE"))
    res = bass_utils.run_bass_kernel_spmd(
        nc,
        in_maps,
        core_ids=list(range(N_CORES)),
        trace=trace,
        **({"tmpdir": "/tmp/ca_prof"} if trace else {}),
    )
    if trace and res.exec_time_ns is not None:
        print(f"HW exec time: {res.exec_time_ns} ns")

    out = np.zeros((B, L, CH), np.float32)
    for c in range(N_CORES):
        arr = np.asarray(res.results[c]["outt"], np.float32)  # [T, 128, 4, R]
        for t, seg in enumerate(per_core[c]):
            if seg is None:
                continue
            b, r0, nv = seg
            rows = arr[t].transpose(2, 1, 0).reshape(R, CH)  # [row, ch]
            out[b, r0: r0 + nv, :] = rows[:nv]
    return out
